# revision 24
# baseline (speedup 1.0000x reference)
"""BEV detection loss kernel for Trainium2 (8 NeuronCores, data-parallel over batch).

Strategy
--------
The reference loss decomposes sparsely:
  * cls_targets is one-hot at <=128 scattered cells/sample, so
      BCE_sum = sum(softplus(z)) - sum(z at scattered (cell,label) positions)
    with softplus(z) = ln(1 + e^z)  (safe in f32 for |z| << 88).
  * box smooth-L1 is masked by reg_masks, nonzero only at the <=128 scattered
    cells, so box_preds is never streamed -- only gathered at 128 rows/sample.

Per core (one sample): stream cls_logits [262144,10] through ACT (exp then
ln(1+u) with row-accumulate), compute scatter indices from gt_boxes on-device,
indirect-DMA gather cls/box rows at those cells, dedup colliding cells via a
PE-transpose equality matrix (reference scatter semantics: set -> last writer
wins for box targets, count distinct cells once), and emit 4 partial scalars
[softplus_sum, bce_correction, box_numerator, positive_count].  The host sums
partials over the 8 cores (the trivial all-reduce of 4 scalars) and forms the
3 losses with the global positive-count normalizer.
"""
import numpy as np

import concourse.bass as bass
import concourse.bacc as bacc
import concourse.tile as tile
from concourse import mybir
from concourse.bass_utils import run_bass_kernel_spmd

# The act-table-load pass maps each ActivationFunctionType to the FIRST table
# set containing it, which puts Exp and Ln in different sets and inserts a
# ~1.3us table switch per exp<->ln alternation.  Hide Exp/Ln from the earlier
# sets (ids must stay stable, so only membership is edited) so both resolve to
# the combined natural_log_exp_and_others set -> exactly one load.
_orig_get_act_tables = bacc.get_activation_tables


def _patched_get_act_tables(arch):
    tables = dict(_orig_get_act_tables(arch))
    exp, ln = mybir.ActivationFunctionType.Exp, mybir.ActivationFunctionType.Ln
    for name, funcs in tables.items():
        if name != "natural_log_exp_and_others" and (exp in funcs or ln in funcs):
            tables[name] = funcs - {exp, ln}
    return tables


bacc.get_activation_tables = _patched_get_act_tables

P = 128            # partitions == boxes per sample
B = 8              # batch == cores
M = 262144         # BEV cells
C = 10             # classes
D = 7              # box dims
F_TOT = M * C // P  # 20480 f32 per partition of one sample's logits
# chunk ladder: small head chunks (ACT starts early), small tail chunks (short
# post-DMA dependency chain); 2 fold rounds on big chunks, 1 on small ones
CHUNKS = [640, 1280] + [1280] * 14 + [640]
FOLDS = [1] + [2] * 14 + [1, 0]   # small head chunk starts ACT early; light tail
NSTREAM = len(CHUNKS)
NCOL = NSTREAM + 3                   # + [bce_corr, box_num, count]

X_MIN = -51.2
INV_RES = 5.0      # 1/0.2
BEV_W = 512.0

F32 = mybir.dt.float32
I32 = mybir.dt.int32
Alu = mybir.AluOpType
Act = mybir.ActivationFunctionType

_BUILT = None
LAST_RESULTS = None
DEBUG_OUTPUTS = False


def _build():
    nc = bacc.Bacc(None, target_bir_lowering=False)

    cls_t = nc.dram_tensor("cls", [M, C], F32, kind="ExternalInput")
    boxp_t = nc.dram_tensor("boxp", [M, D], F32, kind="ExternalInput")
    meta_t = nc.dram_tensor("meta", [P, D + 2], F32, kind="ExternalInput")  # gtb|lbl|msk
    out_t = nc.dram_tensor("out", [1, NCOL], F32, kind="ExternalOutput")

    # all constants in one inline tensor -> one 0.5us DMA off the sync queue
    consts_np = np.concatenate([
        np.eye(P, dtype=np.float32),                                   # iden   [0:128)
        np.tril(np.ones((P, P), np.float32), -1),                      # tril   [128:256)
        np.triu(np.ones((P, P), np.float32), 1),                       # triu   [256:384)
        np.broadcast_to(np.arange(10, dtype=np.float32), (P, 10)),     # io10   [384:394)
        np.arange(P, dtype=np.float32)[:, None],                       # pidx   [394:395)
    ], axis=1)
    consts_c = nc.inline_tensor(np.ascontiguousarray(consts_np), name="constsc")

    cls_stream = cls_t[:].rearrange("(p n) d -> p (n d)", p=P)   # [128, 20480]

    with tile.TileContext(nc) as tc:
        with (
            tc.tile_pool(name="stream", bufs=8) as stp,
            tc.tile_pool(name="work", bufs=1) as wkp,
            tc.tile_pool(name="small", bufs=1) as sm,
            tc.tile_pool(name="psum", bufs=2, space="PSUM") as ps,
        ):
            # vals: per-partition partials, one matmul reduces all columns.
            # cols 0..NSTREAM-1: per-chunk softplus sums (ln accum lands here)
            # col NSTREAM+0: bce correction, +1: box numerator, +2: count
            vals = sm.tile([P, NCOL], F32)

            # ============ small section first (higher scheduler priority;
            # everything here overlaps under the big stream) ============
            meta = sm.tile([P, D + 2], F32)
            nc.gpsimd.dma_start(out=meta[:], in_=meta_t[:])
            gtb = meta[:, 0:D]
            lbl = meta[:, D:D + 1]
            msk = meta[:, D + 1:D + 2]
            consts = sm.tile([P, 395], F32)
            nc.gpsimd.dma_start(out=consts[:], in_=consts_c[:])
            iden = consts[:, 0:128]
            tril = consts[:, 128:256]
            triu = consts[:, 256:384]
            io10 = consts[:, 384:394]
            pidx = consts[:, 394:395]

            half = sm.tile([P, 1], F32)
            nc.vector.memset(half[:], 0.5)
            zero1 = sm.tile([P, 1], F32)
            nc.vector.memset(zero1[:], 0.0)
            ones1 = sm.tile([P, 1], F32)
            nc.vector.memset(ones1[:], 1.0)

            # grid coords: g = floor((x - X_MIN) * INV_RES) via round(r - 0.5)
            def floor_coord(col):
                r = sm.tile([P, 1], F32, name=f"r{col}")
                nc.vector.tensor_scalar(out=r[:], in0=gtb[:, col:col + 1],
                                        scalar1=-X_MIN, scalar2=INV_RES,
                                        op0=Alu.add, op1=Alu.mult)
                rs = sm.tile([P, 1], F32, name=f"rs{col}")
                nc.vector.tensor_scalar(out=rs[:], in0=r[:], scalar1=0.5, scalar2=None,
                                        op0=Alu.subtract)
                gi = sm.tile([P, 1], I32, name=f"gi{col}")
                nc.vector.tensor_copy(out=gi[:], in_=rs[:])      # round-nearest
                gf = sm.tile([P, 1], F32, name=f"gf{col}")
                nc.vector.tensor_copy(out=gf[:], in_=gi[:])
                return gf

            gxf = floor_coord(0)
            gyf = floor_coord(1)
            idxf = sm.tile([P, 1], F32)
            nc.vector.tensor_scalar(out=idxf[:], in0=gyf[:], scalar1=BEV_W,
                                    scalar2=None, op0=Alu.mult)
            nc.vector.tensor_tensor(out=idxf[:], in0=idxf[:], in1=gxf[:], op=Alu.add)
            idx_i = sm.tile([P, 1], I32)
            nc.vector.tensor_copy(out=idx_i[:], in_=idxf[:])

            # valid = (mask > 0.5) & (label >= 0)
            v1 = sm.tile([P, 1], F32)
            nc.vector.tensor_tensor(out=v1[:], in0=msk, in1=half[:], op=Alu.is_gt)
            v2 = sm.tile([P, 1], F32)
            nc.vector.tensor_tensor(out=v2[:], in0=lbl, in1=zero1[:], op=Alu.is_ge)
            valid = sm.tile([P, 1], F32)
            nc.vector.tensor_tensor(out=valid[:], in0=v1[:], in1=v2[:], op=Alu.mult)

            # dedup keys (invalid rows get unique sentinels so they never match)
            sentc = sm.tile([P, 1], F32)
            nc.vector.tensor_scalar(out=sentc[:], in0=pidx, scalar1=float(1 << 22),
                                    scalar2=None, op0=Alu.add)
            sentp = sm.tile([P, 1], F32)
            nc.vector.tensor_scalar(out=sentp[:], in0=pidx, scalar1=float(1 << 23),
                                    scalar2=None, op0=Alu.add)
            # blend: key = sent + valid*(key0 - sent)   (exact: all integers < 2^24)
            ckey = sm.tile([P, 1], F32)
            nc.vector.tensor_tensor(out=ckey[:], in0=idxf[:], in1=sentc[:], op=Alu.subtract)
            nc.vector.tensor_tensor(out=ckey[:], in0=ckey[:], in1=valid[:], op=Alu.mult)
            nc.vector.tensor_tensor(out=ckey[:], in0=ckey[:], in1=sentc[:], op=Alu.add)
            pkey0 = sm.tile([P, 1], F32)
            nc.vector.tensor_scalar(out=pkey0[:], in0=idxf[:], scalar1=16.0,
                                    scalar2=None, op0=Alu.mult)
            nc.vector.tensor_tensor(out=pkey0[:], in0=pkey0[:], in1=lbl, op=Alu.add)
            pkey = sm.tile([P, 1], F32)
            nc.vector.tensor_tensor(out=pkey[:], in0=pkey0[:], in1=sentp[:], op=Alu.subtract)
            nc.vector.tensor_tensor(out=pkey[:], in0=pkey[:], in1=valid[:], op=Alu.mult)
            nc.vector.tensor_tensor(out=pkey[:], in0=pkey[:], in1=sentp[:], op=Alu.add)

            # transpose keys across partitions (PE identity trick)
            ckT_ps = ps.tile([P, P], F32, space="PSUM")
            nc.tensor.transpose(out=ckT_ps[:], in_=ckey[:].to_broadcast([P, P]),
                                identity=iden)
            ckT = sm.tile([P, P], F32)
            nc.vector.tensor_copy(out=ckT[:], in_=ckT_ps[:])
            pkT_ps = ps.tile([P, P], F32, space="PSUM")
            nc.tensor.transpose(out=pkT_ps[:], in_=pkey[:].to_broadcast([P, P]),
                                identity=iden)
            pkT = sm.tile([P, P], F32)
            nc.vector.tensor_copy(out=pkT[:], in_=pkT_ps[:])

            # equality matrices + strict-triangular counts
            eqc = sm.tile([P, P], F32)
            nc.vector.tensor_tensor(out=eqc[:], in0=ckey[:].to_broadcast([P, P]),
                                    in1=ckT[:], op=Alu.is_equal)
            eqp = sm.tile([P, P], F32)
            nc.vector.tensor_tensor(out=eqp[:], in0=pkey[:].to_broadcast([P, P]),
                                    in1=pkT[:], op=Alu.is_equal)
            scrP = sm.tile([P, P], F32)
            nlt = sm.tile([P, 1], F32)
            nc.vector.tensor_tensor(out=scrP[:], in0=eqc[:], in1=tril, op=Alu.mult)
            nc.vector.tensor_reduce(out=nlt[:], in_=scrP[:], axis=mybir.AxisListType.X,
                                    op=Alu.add)
            ngt = sm.tile([P, 1], F32)
            nc.vector.tensor_tensor(out=scrP[:], in0=eqc[:], in1=triu, op=Alu.mult)
            nc.vector.tensor_reduce(out=ngt[:], in_=scrP[:], axis=mybir.AxisListType.X,
                                    op=Alu.add)
            plt = sm.tile([P, 1], F32)
            nc.vector.tensor_tensor(out=scrP[:], in0=eqp[:], in1=tril, op=Alu.mult)
            nc.vector.tensor_reduce(out=plt[:], in_=scrP[:], axis=mybir.AxisListType.X,
                                    op=Alu.add)
            firstc = sm.tile([P, 1], F32)
            nc.vector.tensor_tensor(out=firstc[:], in0=nlt[:], in1=zero1[:], op=Alu.is_equal)
            lastc = sm.tile([P, 1], F32)
            nc.vector.tensor_tensor(out=lastc[:], in0=ngt[:], in1=zero1[:], op=Alu.is_equal)
            firstp = sm.tile([P, 1], F32)
            nc.vector.tensor_tensor(out=firstp[:], in0=plt[:], in1=zero1[:], op=Alu.is_equal)

            # ---------------- indirect gathers ----------------
            zrow = sm.tile([P, C], F32)
            nc.gpsimd.indirect_dma_start(
                out=zrow[:], out_offset=None, in_=cls_t[:],
                in_offset=bass.IndirectOffsetOnAxis(ap=idx_i[:, :1], axis=0))
            bp = sm.tile([P, D], F32)
            nc.gpsimd.indirect_dma_start(
                out=bp[:], out_offset=None, in_=boxp_t[:],
                in_offset=bass.IndirectOffsetOnAxis(ap=idx_i[:, :1], axis=0))

            # z at (cell,label): one-hot dot gathered row
            onehot = sm.tile([P, C], F32)
            nc.vector.tensor_tensor(out=onehot[:], in0=io10,
                                    in1=lbl.to_broadcast([P, C]), op=Alu.is_equal)
            scrC = sm.tile([P, C], F32)
            z_i = sm.tile([P, 1], F32)
            nc.vector.tensor_tensor(out=scrC[:], in0=onehot[:], in1=zrow[:], op=Alu.mult)
            nc.vector.tensor_reduce(out=z_i[:], in_=scrC[:], axis=mybir.AxisListType.X,
                                    op=Alu.add)

            # smooth-L1 row sums: d = bp - gt;  sl1 = (|d|<1 ? 0.5 d^2 : |d|-0.5)
            dtile = sm.tile([P, D], F32)
            nc.vector.tensor_tensor(out=dtile[:], in0=bp[:], in1=gtb, op=Alu.subtract)
            absd = sm.tile([P, D], F32)
            nc.vector.scalar_tensor_tensor(out=absd[:], in0=dtile[:], scalar=-1.0,
                                           in1=dtile[:], op0=Alu.mult, op1=Alu.max)
            quad = sm.tile([P, D], F32)
            nc.vector.tensor_tensor(out=quad[:], in0=dtile[:], in1=dtile[:], op=Alu.mult)
            nc.vector.tensor_scalar(out=quad[:], in0=quad[:], scalar1=0.5, scalar2=None,
                                    op0=Alu.mult)
            lin = sm.tile([P, D], F32)
            nc.vector.tensor_scalar(out=lin[:], in0=absd[:], scalar1=0.5, scalar2=None,
                                    op0=Alu.subtract)
            mlt = sm.tile([P, D], F32)
            nc.vector.tensor_tensor(out=mlt[:], in0=absd[:],
                                    in1=ones1[:].to_broadcast([P, D]), op=Alu.is_lt)
            # sl1 = lin + m*(quad - lin)
            sl1 = sm.tile([P, D], F32)
            nc.vector.tensor_tensor(out=sl1[:], in0=quad[:], in1=lin[:], op=Alu.subtract)
            nc.vector.tensor_tensor(out=sl1[:], in0=sl1[:], in1=mlt[:], op=Alu.mult)
            nc.vector.tensor_tensor(out=sl1[:], in0=sl1[:], in1=lin[:], op=Alu.add)
            sl1s = sm.tile([P, 1], F32)
            nc.vector.tensor_reduce(out=sl1s[:], in_=sl1[:], axis=mybir.AxisListType.X,
                                    op=Alu.add)

            # partial columns (written straight into vals)
            corr = sm.tile([P, 1], F32)
            nc.vector.tensor_tensor(out=corr[:], in0=valid[:], in1=firstp[:], op=Alu.mult)
            nc.vector.tensor_tensor(out=vals[:, NSTREAM:NSTREAM + 1], in0=corr[:],
                                    in1=z_i[:], op=Alu.mult)
            bnum = sm.tile([P, 1], F32)
            nc.vector.tensor_tensor(out=bnum[:], in0=valid[:], in1=lastc[:], op=Alu.mult)
            nc.vector.tensor_tensor(out=vals[:, NSTREAM + 1:NSTREAM + 2], in0=bnum[:],
                                    in1=sl1s[:], op=Alu.mult)
            nc.vector.tensor_tensor(out=vals[:, NSTREAM + 2:NSTREAM + 3], in0=valid[:],
                                    in1=firstc[:], op=Alu.mult)

            # ============ streaming softplus sum ============
            # sum softplus(z) = sum ln(1+e^z) = sum ln PI(1+e^z_i): per chunk
            # ACT exp (in place), DVE +1 (2x tensor_scalar), one pairwise fold
            # (halves product, f32-safe: terms <= 1+e^6), ACT ln over F/2 with
            # row-accumulate into vals[:, k].  First chunks are smaller so ACT
            # starts early.
            FMAX = max(CHUNKS)
            lnsink = wkp.tile([P, FMAX], F32, name="lnsink")
            off = 0
            pending = None   # software-pipeline: emit chunk k's ln after exp k+1
            for k, (Fk, rk) in enumerate(zip(CHUNKS, FOLDS)):
                t = stp.tile([P, FMAX], F32, name="t")
                nc.sync.dma_start(out=t[:, :Fk], in_=cls_stream[:, off:off + Fk])
                off += Fk
                nc.scalar.activation(out=t[:, :Fk], in_=t[:, :Fk], func=Act.Exp)
                if pending is not None:
                    pt, pw, pk, pbias = pending
                    nc.scalar.activation(out=lnsink[:, :pw], in_=pt[:, :pw], func=Act.Ln,
                                         bias=pbias, accum_out=vals[:, pk:pk + 1])
                if rk == 0:
                    pending = (t, Fk, k, 1.0)
                    continue
                nc.vector.tensor_scalar(out=t[:, :Fk], in0=t[:, :Fk], scalar1=1.0,
                                        scalar2=None, op0=Alu.add)
                w = Fk
                for _ in range(rk):
                    h = w // 2
                    nc.vector.tensor_tensor(out=t[:, :h], in0=t[:, :h], in1=t[:, h:w],
                                            op=Alu.mult)
                    w = h
                pending = (t, w, k, 0.0)
            pt, pw, pk, pbias = pending
            nc.scalar.activation(out=lnsink[:, :pw], in_=pt[:, :pw], func=Act.Ln,
                                 bias=pbias, accum_out=vals[:, pk:pk + 1])

            # ============ finale: one matmul reduces all partials ============
            mm = ps.tile([1, NCOL], F32, space="PSUM")
            nc.tensor.matmul(out=mm[:], lhsT=ones1[:], rhs=vals[:], start=True, stop=True)
            outv = sm.tile([1, NCOL], F32)
            nc.vector.tensor_copy(out=outv[:], in_=mm[:])
            nc.sync.dma_start(out=out_t[:], in_=outv[:])

            if DEBUG_OUTPUTS:
                for nm, tl in [("d_idx", idxf), ("d_valid", valid), ("d_firstp", firstp),
                               ("d_lastc", lastc), ("d_firstc", firstc), ("d_z", z_i),
                               ("d_sl1s", sl1s), ("d_pkey", pkey)]:
                    dt = nc.dram_tensor(nm, [P, 1], F32, kind="ExternalOutput")
                    cp = sm.tile([P, 1], F32, name=f"cp{nm}")
                    nc.vector.tensor_copy(out=cp[:], in_=tl[:])
                    nc.sync.dma_start(out=dt[:], in_=cp[:])
                dzr = nc.dram_tensor("d_zrow", [P, C], F32, kind="ExternalOutput")
                cpz = sm.tile([P, C], F32)
                nc.vector.tensor_copy(out=cpz[:], in_=zrow[:])
                nc.sync.dma_start(out=dzr[:], in_=cpz[:])
                dbp = nc.dram_tensor("d_bp", [P, D], F32, kind="ExternalOutput")
                cpb = sm.tile([P, D], F32)
                nc.vector.tensor_copy(out=cpb[:], in_=bp[:])
                nc.sync.dma_start(out=dbp[:], in_=cpb[:])

    nc.finalize()
    return nc


def kernel(cls_logits, box_preds, gt_boxes, gt_labels, gt_masks):
    global _BUILT, LAST_RESULTS
    if _BUILT is None:
        _BUILT = _build()
    nc = _BUILT

    cls_logits = np.ascontiguousarray(cls_logits, dtype=np.float32)
    box_preds = np.ascontiguousarray(box_preds, dtype=np.float32)
    gt_boxes = np.ascontiguousarray(gt_boxes, dtype=np.float32)
    lblf = np.asarray(gt_labels).astype(np.float32).reshape(B, P, 1)
    mskf = np.asarray(gt_masks).astype(np.float32).reshape(B, P, 1)

    meta = np.concatenate([gt_boxes, lblf, mskf], axis=2)  # [B, P, 9]
    in_maps = [
        {"cls": cls_logits[c], "boxp": box_preds[c], "meta": meta[c]}
        for c in range(B)
    ]
    LAST_RESULTS = run_bass_kernel_spmd(nc, in_maps, list(range(B)))
    parts = np.stack([LAST_RESULTS.results[c]["out"][0] for c in range(B)])  # [8,NCOL]
    tot = parts.astype(np.float64).sum(0)
    s_soft = tot[:NSTREAM].sum()
    corr, boxnum, cnt = tot[NSTREAM], tot[NSTREAM + 1], tot[NSTREAM + 2]
    cls_loss = (s_soft - corr) / float(B * M)
    box_loss = boxnum / (cnt + 1e-6)
    total = cls_loss + box_loss
    return np.array([total, cls_loss, box_loss], dtype=np.float32)
